# revision 8
# baseline (speedup 1.0000x reference)
"""MARNN (nn_MARNN_70815420776936) Trainium2 Bass kernel.

Data-parallel over batch: B=256 -> 8 NeuronCores x 32.
Per core, per timestep (contraction on partitions, batch=32 as the matmul
stationary/M dim, weights moving as rhs):

  head  = [x_t, c] @ fc_w.T            (fp32 -- feeds argmax, precision-critical)
  amax  = argmax(head + gumbel)        (DVE max/max_index)
  h     = hmem[b, amax_b]              (indirect DMA gather from DRAM)
  g     = sigmoid([x, c, h] @ W1)      (float32r weights)
  pre   = [x, c*g_c, h*g_h] @ W        (float32r weights, cols permuted [i,f,j,o,om])
  c'    = tanh(c*sig(f+1) + sig(i)*tanh(j))
  out_t = [c'*sig(o), h*sig(om)]
  wv    = c' @ trans_w.T               (fp32)
  hmem[b, slot_b] = wv_b               (indirect DMA scatter; slot = t if t<128 else amax_b)

float32r is fp32 with operands RNE-rounded to 11 mantissa bits at 4x the
fp32 matmul rate; weights are pre-rounded on the host.  The head matmul is
kept fp32 so the argmax decisions match an fp32 reference (measured: 0
argmax flips, end-to-end rel_l2 ~1e-4).
"""
import numpy as np

X, H, R, M = 256, 512, 128, 128
T_FULL, B_FULL = 256, 256
NCORES = 8
BL = B_FULL // NCORES          # 32 batch rows per core
SC = 8                         # timesteps per IO chunk
F_BIAS = 1.0


def _round_f32r(x):
    x = np.ascontiguousarray(x, dtype=np.float32)
    xi = x.view(np.uint32).astype(np.uint64)
    drop = 12  # keep 11 explicit mantissa bits (matches HW f32r rounding)
    xq = ((xi + (1 << (drop - 1))) >> drop) << drop
    return xq.astype(np.uint32).view(np.float32)


def _split_multiwaits(nc):
    """This walrus build accepts one sync-wait per instruction; split extras
    into single-wait Drain carriers on the same engine."""
    import concourse.mybir as mybir
    ctr = 0
    for f in nc.m.functions:
        for b in f.blocks:
            out = []
            changed = False
            for inst in b.instructions:
                si = inst.sync_info
                if si is not None and len(si.on_wait) > 1:
                    waits = list(si.on_wait)
                    for w in waits[:-1]:
                        ctr += 1
                        d = mybir.InstDrain(
                            name=f"I-mwfix{ctr}", ins=[], outs=[],
                            bass_is_fusable=False,
                        )
                        d.engine = inst.engine
                        d.sync_info = mybir.SyncInfo(on_wait=[w], on_update=[])
                        out.append(d)
                    inst.sync_info = mybir.SyncInfo(
                        on_wait=[waits[-1]], on_update=list(si.on_update)
                    )
                    changed = True
                out.append(inst)
            if changed:
                b.instructions = out
    return ctr


def _build(T, bias_on, fcb_on):
    import concourse.bass as bass
    import concourse.mybir as mybir
    from concourse.tile import TileContext
    from concourse.masks import make_identity

    dt = mybir.dt
    AF = mybir.ActivationFunctionType
    OP = mybir.AluOpType

    nc = bass.Bass(target_bir_lowering=False)

    # extra scalar constants used by activation bias/scale
    for cval in (1e-20, -1.0, float(F_BIAS)):
        if (dt.float32, cval) not in nc.const_aps.aps:
            ct = nc.alloc_sbuf_tensor(f"const-f32-{cval}", [128, 1], dt.float32)
            nc.gpsimd.memset(ct.ap(), cval)
            nc.const_aps.aps[(dt.float32, cval)] = ct.ap()
    nc.all_engine_barrier()

    NCHUNK = (T + SC - 1) // SC
    xp = nc.declare_dram_parameter("xp", [NCHUNK, 128, SC * 2 * BL], dt.float32,
                                   isOutput=False)
    np_ = nc.declare_dram_parameter("noise", [NCHUNK, BL, SC * M], dt.float32,
                                    isOutput=False)
    fcT = nc.declare_dram_parameter("fcT", [128, 6 * 128], dt.float32, isOutput=False)
    w1 = nc.declare_dram_parameter("w1", [128, 7 * (R + H)], dt.float32r, isOutput=False)
    wf = nc.declare_dram_parameter("wf", [128, 7 * (R + 4 * H)], dt.float32r,
                                   isOutput=False)
    trT = nc.declare_dram_parameter("trT", [128, 4 * R], dt.float32, isOutput=False)
    hmem0 = nc.declare_dram_parameter("hmem0", [BL * M, R], dt.float32, isOutput=False)
    c0T = nc.declare_dram_parameter("c0T", [128, 4], dt.float32, isOutput=False)
    c0bm = nc.declare_dram_parameter("c0bm", [BL, H], dt.float32, isOutput=False)
    rowt = nc.declare_dram_parameter("rowt", [BL, M + 1], dt.uint32, isOutput=False)
    if fcb_on:
        fcbrep = nc.declare_dram_parameter("fcbrep", [BL, M], dt.float32, isOutput=False)
    if bias_on:
        b1p = nc.declare_dram_parameter("b1p", [1, R + H], dt.float32r, isOutput=False)
        bfp = nc.declare_dram_parameter("bfp", [1, R + 4 * H], dt.float32r, isOutput=False)
        btr = nc.declare_dram_parameter("btr", [1, R], dt.float32, isOutput=False)
    out_d = nc.declare_dram_parameter("out", [NCHUNK, BL, SC * (H + R)], dt.float32,
                                      isOutput=True)

    hmem = nc.dram_tensor("hmem", [BL * M, R], dt.float32)

    OUTW = H + R

    with TileContext(nc) as tc:
        with (
            tc.tile_pool(name="wpool", bufs=1) as wp,
            tc.tile_pool(name="state", bufs=2) as stp,
            tc.tile_pool(name="io", bufs=2) as iop,
            tc.tile_pool(name="io1", bufs=1) as iop1,
            tc.tile_pool(name="work", bufs=2) as wk,
            tc.tile_pool(name="ps", bufs=8, space="PSUM") as ps,
        ):
            fcT_t = wp.tile([128, 6 * 128], dt.float32, tag="fcT")
            nc.sync.dma_start(out=fcT_t[:], in_=fcT[:])
            w1_t = wp.tile([128, 7 * (R + H)], dt.float32r, tag="w1")
            nc.sync.dma_start(out=w1_t[:], in_=w1[:])
            wf_t = wp.tile([128, 7 * (R + 4 * H)], dt.float32r, tag="wf")
            nc.sync.dma_start(out=wf_t[:], in_=wf[:])
            trT_t = wp.tile([128, 4 * R], dt.float32, tag="trT")
            nc.sync.dma_start(out=trT_t[:], in_=trT[:])
            rowt_t = wp.tile([BL, M + 1], dt.uint32, tag="rowt")
            nc.sync.dma_start(out=rowt_t[:], in_=rowt[:])
            ident = wp.tile([128, 128], dt.float32, tag="ident")
            make_identity(nc, ident[:])
            if fcb_on:
                fcb_t = wp.tile([BL, M], dt.float32, tag="fcb")
                nc.sync.dma_start(out=fcb_t[:], in_=fcbrep[:])
            if bias_on:
                ones_t = wp.tile([1, BL], dt.float32, tag="ones")
                nc.vector.memset(ones_t[:], 1.0)
                ones_r = wp.tile([1, BL], dt.float32r, tag="ones_r")
                nc.vector.memset(ones_r[:], 1.0)
                b1_t = wp.tile([1, R + H], dt.float32r, tag="b1")
                nc.sync.dma_start(out=b1_t[:], in_=b1p[:])
                bf_t = wp.tile([1, R + 4 * H], dt.float32r, tag="bf")
                nc.sync.dma_start(out=bf_t[:], in_=bfp[:])
                btr_t = wp.tile([1, R], dt.float32, tag="btr")
                nc.sync.dma_start(out=btr_t[:], in_=btr[:])

            cat_prev = stp.tile([128, 5 * BL], dt.float32, tag="catT")
            c0T_t = wp.tile([128, 4], dt.float32, tag="c0T")
            nc.sync.dma_start(out=c0T_t[:], in_=c0T[:])
            for k in range(4):
                nc.vector.tensor_copy(
                    cat_prev[:, k * BL:(k + 1) * BL],
                    c0T_t[:, k:k + 1].to_broadcast([128, BL]),
                )
            catr_prev = stp.tile([128, 5 * BL], dt.float32r, tag="catTr")
            nc.vector.tensor_copy(catr_prev[:, 0:4 * BL], cat_prev[:, 0:4 * BL])
            cbm_prev = stp.tile([BL, H], dt.float32, tag="cbm")
            nc.sync.dma_start(out=cbm_prev[:], in_=c0bm[:])

            # hmem init on the same SWDGE queue as the indirect DMAs (FIFO)
            nc.gpsimd.dma_start(out=hmem[:], in_=hmem0[:])

            for ch in range(NCHUNK):
                s0 = ch * SC
                ns = min(SC, T - s0)
                xb = iop.tile([128, SC * 2 * BL], dt.float32, tag="xb")
                nc.sync.dma_start(out=xb[:], in_=xp[ch])
                xbr = iop.tile([128, SC * 2 * BL], dt.float32r, tag="xbr")
                nc.vector.tensor_copy(xbr[:, :ns * 2 * BL], xb[:, :ns * 2 * BL])
                nb = iop.tile([BL, SC * M], dt.float32, tag="nb")
                nc.sync.dma_start(out=nb[:], in_=np_[ch])
                # gl2 = log(1e-20 - log(noise + 1e-20));  logits = head - gl2
                l1 = iop1.tile([BL, SC * M], dt.float32, tag="l1")
                nc.scalar.activation(l1[:, :ns * M], nb[:, :ns * M], AF.Ln, bias=1e-20)
                gl2 = iop.tile([BL, SC * M], dt.float32, tag="gl2")
                nc.scalar.activation(gl2[:, :ns * M], l1[:, :ns * M], AF.Ln,
                                     bias=1e-20, scale=-1.0)
                ob = iop.tile([BL, SC * OUTW], dt.float32, tag="ob")

                for s in range(ns):
                    t = s0 + s
                    xs0 = xb[:, (2 * s) * BL:(2 * s + 1) * BL]
                    xs1 = xb[:, (2 * s + 1) * BL:(2 * s + 2) * BL]
                    xr0 = xbr[:, (2 * s) * BL:(2 * s + 1) * BL]
                    xr1 = xbr[:, (2 * s + 1) * BL:(2 * s + 2) * BL]

                    # ---- head (fp32) ----
                    p_h = ps.tile([BL, M], dt.float32, tag="ps")
                    head_lhs = [xs0, xs1] + \
                        [cat_prev[:, k * BL:(k + 1) * BL] for k in range(4)]
                    for k, lhs in enumerate(head_lhs):
                        nc.tensor.matmul(
                            p_h[:], lhsT=lhs, rhs=fcT_t[:, k * 128:(k + 1) * 128],
                            start=(k == 0), stop=(k == 5),
                        )
                    logits = wk.tile([BL, M], dt.float32, tag="logits")
                    nc.vector.tensor_tensor(
                        logits[:], p_h[:], gl2[:, s * M:(s + 1) * M], op=OP.subtract
                    )
                    if fcb_on:
                        nc.vector.tensor_tensor(
                            logits[:], logits[:], fcb_t[:], op=OP.add
                        )
                    mx8 = wk.tile([BL, 8], dt.float32, tag="mx8")
                    nc.vector.max(mx8[:], logits[:])
                    mi8 = wk.tile([BL, 8], dt.uint32, tag="mi8")
                    nc.vector.max_index(mi8[:], mx8[:], logits[:])
                    ridx = wk.tile([BL, 1], dt.uint32, tag="ridx")
                    nc.vector.tensor_tensor(
                        ridx[:], mi8[:, 0:1], rowt_t[:, 0:1], op=OP.add
                    )
                    wrow = rowt_t[:, t + 1:t + 2] if t < M else ridx[:, 0:1]

                    # ---- gather h_entry = hmem[b, amax_b] ----
                    hent = wk.tile([BL, R], dt.float32, tag="hent")
                    nc.gpsimd.indirect_dma_start(
                        out=hent[:], out_offset=None, in_=hmem[:],
                        in_offset=bass.IndirectOffsetOnAxis(ap=ridx[:, 0:1], axis=0),
                    )
                    p_hT = ps.tile([128, BL], dt.float32, tag="ps")
                    nc.tensor.transpose(p_hT[:], hent[:], ident[:BL, :BL])
                    cat = stp.tile([128, 5 * BL], dt.float32, tag="catT")
                    nc.vector.tensor_copy(cat[:, 4 * BL:5 * BL], p_hT[:])
                    catr = stp.tile([128, 5 * BL], dt.float32r, tag="catTr")
                    nc.vector.tensor_copy(catr[:, 4 * BL:5 * BL], p_hT[:])

                    # ---- gates1 ----
                    p_g1 = ps.tile([BL, 512], dt.float32, tag="ps")
                    p_g2 = ps.tile([BL, R + H - 512], dt.float32, tag="ps")
                    g1_lhs = [xr0, xr1] + \
                        [catr_prev[:, k * BL:(k + 1) * BL] for k in range(4)] + \
                        [catr[:, 4 * BL:5 * BL]]
                    nmm = 7 + (1 if bias_on else 0)
                    for k in range(nmm):
                        if k < 7:
                            lhs = g1_lhs[k]
                            rhs0 = w1_t[:, k * 640:k * 640 + 512]
                            rhs1 = w1_t[:, k * 640 + 512:(k + 1) * 640]
                        else:
                            lhs, rhs0, rhs1 = ones_r[:], b1_t[:, 0:512], b1_t[:, 512:640]
                        nc.tensor.matmul(p_g1[:], lhsT=lhs, rhs=rhs0,
                                         start=(k == 0), stop=(k == nmm - 1))
                        nc.tensor.matmul(p_g2[:], lhsT=lhs, rhs=rhs1,
                                         start=(k == 0), stop=(k == nmm - 1))
                    gsig = wk.tile([BL, R + H], dt.float32, tag="gsig")
                    nc.scalar.activation(gsig[:, 0:512], p_g1[:], AF.Sigmoid)
                    nc.scalar.activation(gsig[:, 512:640], p_g2[:], AF.Sigmoid)
                    gated = wk.tile([128, 5 * BL], dt.float32r, tag="gated")
                    for k in range(5):
                        p_gT = ps.tile([128, BL], dt.float32, tag="ps")
                        nc.tensor.transpose(
                            p_gT[:], gsig[:, k * 128:(k + 1) * 128], ident[:BL, :BL]
                        )
                        src = cat_prev[:, k * BL:(k + 1) * BL] if k < 4 \
                            else cat[:, 4 * BL:5 * BL]
                        nc.vector.tensor_tensor(
                            gated[:, k * BL:(k + 1) * BL], src, p_gT[:], op=OP.mult
                        )

                    # ---- pre, columns [i f j o om] ----
                    GN = [512, 512, 512, 512, 128]
                    p_pre = [
                        ps.tile([BL, n], dt.float32, tag="ps", name=f"p_pre{t}_{gi}")
                        for gi, n in enumerate(GN)
                    ]
                    pre_lhs = [xr0, xr1] + \
                        [gated[:, k * BL:(k + 1) * BL] for k in range(5)]
                    nmm = 7 + (1 if bias_on else 0)
                    for k in range(nmm):
                        off = 0
                        for gi, n in enumerate(GN):
                            if k < 7:
                                lhs = pre_lhs[k]
                                rhs = wf_t[:, k * 2176 + off:k * 2176 + off + n]
                            else:
                                lhs, rhs = ones_r[:], bf_t[:, off:off + n]
                            nc.tensor.matmul(p_pre[gi][:], lhsT=lhs, rhs=rhs,
                                             start=(k == 0), stop=(k == nmm - 1))
                            off += n
                    sig_i = wk.tile([BL, H], dt.float32, tag="sig_i")
                    nc.scalar.activation(sig_i[:], p_pre[0][:], AF.Sigmoid)
                    sig_f = wk.tile([BL, H], dt.float32, tag="sig_f")
                    nc.scalar.activation(sig_f[:], p_pre[1][:], AF.Sigmoid, bias=F_BIAS)
                    tan_j = wk.tile([BL, H], dt.float32, tag="tan_j")
                    nc.scalar.activation(tan_j[:], p_pre[2][:], AF.Tanh)
                    sig_o = wk.tile([BL, H], dt.float32, tag="sig_o")
                    nc.scalar.activation(sig_o[:], p_pre[3][:], AF.Sigmoid)
                    sig_om = wk.tile([BL, R], dt.float32, tag="sig_om")
                    nc.scalar.activation(sig_om[:], p_pre[4][:], AF.Sigmoid)

                    # ---- new c ----
                    t1 = wk.tile([BL, H], dt.float32, tag="t1")
                    nc.vector.tensor_tensor(t1[:], sig_i[:], tan_j[:], op=OP.mult)
                    t2 = wk.tile([BL, H], dt.float32, tag="t2")
                    nc.vector.tensor_tensor(t2[:], cbm_prev[:], sig_f[:], op=OP.mult)
                    t3 = wk.tile([BL, H], dt.float32, tag="t3")
                    nc.vector.tensor_tensor(t3[:], t1[:], t2[:], op=OP.add)
                    cbm = stp.tile([BL, H], dt.float32, tag="cbm")
                    nc.scalar.activation(cbm[:], t3[:], AF.Tanh)
                    for k in range(4):
                        p_cT = ps.tile([128, BL], dt.float32, tag="ps")
                        nc.tensor.transpose(
                            p_cT[:], cbm[:, k * 128:(k + 1) * 128], ident[:BL, :BL]
                        )
                        nc.vector.tensor_copy(cat[:, k * BL:(k + 1) * BL], p_cT[:])
                    nc.vector.tensor_copy(catr[:, 0:4 * BL], cat[:, 0:4 * BL])

                    # ---- outputs ----
                    nc.vector.tensor_tensor(
                        ob[:, s * OUTW:s * OUTW + H], cbm[:], sig_o[:], op=OP.mult
                    )
                    nc.vector.tensor_tensor(
                        ob[:, s * OUTW + H:(s + 1) * OUTW], hent[:], sig_om[:],
                        op=OP.mult,
                    )

                    # ---- wv + scatter ----
                    p_wv = ps.tile([BL, R], dt.float32, tag="ps")
                    nmm = 4 + (1 if bias_on else 0)
                    for k in range(nmm):
                        if k < 4:
                            lhs = cat[:, k * BL:(k + 1) * BL]
                            rhs = trT_t[:, k * R:(k + 1) * R]
                        else:
                            lhs, rhs = ones_t[:], btr_t[:]
                        nc.tensor.matmul(p_wv[:], lhsT=lhs, rhs=rhs,
                                         start=(k == 0), stop=(k == nmm - 1))
                    wv = wk.tile([BL, R], dt.float32, tag="wv")
                    nc.vector.tensor_copy(wv[:], p_wv[:])
                    nc.gpsimd.indirect_dma_start(
                        out=hmem[:],
                        out_offset=bass.IndirectOffsetOnAxis(ap=wrow, axis=0),
                        in_=wv[:], in_offset=None,
                    )

                    cat_prev = cat
                    catr_prev = catr
                    cbm_prev = cbm

                nc.sync.dma_start(out=out_d[ch], in_=ob[:])
    return nc


def _prep_core_inputs(inputs, core, T):
    W_full = np.asarray(inputs["W_full"], np.float32)
    bias = np.asarray(inputs["bias"], np.float32)
    W_full1 = np.asarray(inputs["W_full1"], np.float32)
    bias1 = np.asarray(inputs["bias1"], np.float32)
    fc_w = np.asarray(inputs["fc_w"], np.float32)
    fc_b = np.asarray(inputs["fc_b"], np.float32)
    trans_w = np.asarray(inputs["trans_w"], np.float32)
    trans_b = np.asarray(inputs["trans_b"], np.float32)
    c_bias = np.asarray(inputs["c_bias"], np.float32)
    hmem_bias = np.asarray(inputs["hmem_bias"], np.float32)

    sl = slice(core * BL, (core + 1) * BL)
    xs = np.asarray(inputs["x"][:T, sl, :], np.float32)
    ns = np.asarray(inputs["noise"][:T, sl, :], np.float32)

    perm = np.concatenate([
        np.arange(0, H), np.arange(2 * H, 3 * H), np.arange(H, 2 * H),
        np.arange(3 * H, 4 * H), np.arange(4 * H, 4 * H + R),
    ])
    Wp = W_full[:, perm]
    bp = bias[perm]

    c0 = np.tanh(c_bias[0].astype(np.float32))
    rowt = np.zeros((BL, M + 1), np.uint32)
    rowt[:, 0] = np.arange(BL, dtype=np.uint32) * M
    for t in range(M):
        rowt[:, t + 1] = rowt[:, 0] + t

    NCHUNK = (T + SC - 1) // SC
    assert T % SC == 0
    # x: [T, BL, X] -> chunks [NCHUNK, 128, SC*2*BL] with free order (s, k, b)
    xq = xs.transpose(0, 2, 1).reshape(NCHUNK, SC, 2, 128, BL)
    xq = np.ascontiguousarray(xq.transpose(0, 3, 1, 2, 4)).reshape(NCHUNK, 128, SC * 2 * BL)
    nq = np.ascontiguousarray(
        ns.reshape(NCHUNK, SC, BL, M).transpose(0, 2, 1, 3)
    ).reshape(NCHUNK, BL, SC * M)

    def ktile(w):  # [K, N] -> [128, (k n)] contiguous
        k = w.shape[0] // 128
        return np.ascontiguousarray(
            w.reshape(k, 128, w.shape[1]).transpose(1, 0, 2)
        ).reshape(128, k * w.shape[1])

    d = {
        "xp": xq,
        "noise": nq,
        "fcT": ktile(np.ascontiguousarray(fc_w.T)),
        "w1": _round_f32r(ktile(W_full1)),
        "wf": _round_f32r(ktile(np.ascontiguousarray(Wp))),
        "trT": ktile(np.ascontiguousarray(trans_w.T)),
        "hmem0": np.ascontiguousarray(
            np.broadcast_to(hmem_bias[0], (BL, M, R)).reshape(BL * M, R)
        ).astype(np.float32),
        "c0T": np.ascontiguousarray(c0.reshape(4, 128).T),
        "c0bm": np.ascontiguousarray(np.broadcast_to(c0, (BL, H))).astype(np.float32),
        "rowt": rowt,
    }
    bias_on = bool(np.any(bias) or np.any(bias1) or np.any(trans_b))
    fcb_on = bool(np.any(fc_b))
    if fcb_on:
        d["fcbrep"] = np.ascontiguousarray(np.broadcast_to(fc_b, (BL, M))).astype(np.float32)
    if bias_on:
        d["b1p"] = _round_f32r(bias1.reshape(1, R + H))
        d["bfp"] = _round_f32r(bp.reshape(1, R + 4 * H))
        d["btr"] = trans_b.reshape(1, R).astype(np.float32)
    return d, bias_on, fcb_on


def run(inputs, T=T_FULL, cores=NCORES, trace=False, tmpdir=None):
    from concourse.bass_utils import run_bass_kernel_spmd

    in_maps = []
    bias_on = fcb_on = None
    for c in range(cores):
        d, b, fb = _prep_core_inputs(inputs, c, T)
        in_maps.append(d)
        bias_on, fcb_on = b, fb
    nc = _build(T, bias_on, fcb_on)
    _split_multiwaits(nc)
    r = run_bass_kernel_spmd(nc, in_maps, list(range(cores)), trace=trace,
                             tmpdir=tmpdir)
    nchunk = (T + SC - 1) // SC
    outs = []
    for c in range(cores):
        o = r.results[c]["out"].reshape(nchunk, BL, SC, H + R)
        outs.append(o.transpose(0, 2, 1, 3).reshape(T, BL, H + R))
    full = np.concatenate(outs, axis=1)
    return full, r


def kernel(**inputs):
    full, _ = run(inputs)
    return full
